# revision 1
# baseline (speedup 1.0000x reference)
"""Trainium2 Bass kernel for ConditioningGNN (2-layer GCN + MLP head).

Math (per reference):
  src,dst,norm = gcn_norm(edge_index)  with self-loops, norm = dinv[src]*dinv[dst]
  h0 = [x | y[batch]]
  h1 = relu(A' @ (h0 W1) + b1);  h2 = relu(A' @ (h1 W2) + b2)
  st = relu(h2 W3 + b3) W4 + b4;  return split(st)

Device decomposition (8 NeuronCores, one SPMD program):
  * Nodes padded to 100352 = 8*12544; core c owns dst rows [c*12544,(c+1)*12544).
  * A' @ M = D^-1/2 (A+I) D^-1/2 M. Table rows T'[n] = (h W)[n] * dinv[n] are
    built per-core for owned rows and AllGathered into a full table.
  * Aggregation is feature-major on PE: for each 128-node dst block, PSUM
    accumulates msg_g.T @ onehot_g over groups of 128 gathered edges; the
    self-loop term is one extra matmul lhsT=T'_block, rhs=I (gives T'_block.T).
  * Per-edge gathers use the int16 dma_gather, so the table is split in 4
    chunks of 25088 rows; edges are bucketed by (dst block, src chunk) with a
    fixed group quota Q per cell so all cores share one program.
  * Epilogue per block: S_T * dinv_bcast (DVE) -> relu(.+b) (ACT) stays
    feature-major; T_next = h_T.T @ W (PE) is node-major for the next table.
  * Head runs per block entirely feature-major; output is st.T slices.
"""
import sys
sys.path.insert(0, '/opt/trn_rl_repo')
sys.path.insert(0, '/opt/pypackages')

import numpy as np

N_NODES = 100000
N_PAD = 100352
NCORES = 8
PER_CORE = N_PAD // NCORES          # 12544
NB = PER_CORE // 128                # 98 dst blocks per core
NCHUNK = 4
CHUNK = N_PAD // NCHUNK             # 25088
HID = 128
X_DIM = 4
Y_DIM = 2
IN_DIM = X_DIM + Y_DIM
GPC = 20                            # groups per gather call
IB = 8                              # groups per indicator build
SINGLE_PACKET = False               # >64 descriptors/engine per packet hangs HW


# ----------------------------------------------------------------- host prep

def _prep(x, edge_index, y, batch):
    """Bucket edges, build per-core device arrays. Returns (Q, in_maps_extra)."""
    src = np.asarray(edge_index[0], dtype=np.int64)
    dst = np.asarray(edge_index[1], dtype=np.int64)
    E = src.shape[0]

    deg = np.bincount(dst, minlength=N_NODES).astype(np.float64) + 1.0
    dinv = (1.0 / np.sqrt(deg)).astype(np.float32)
    dinv_pad = np.zeros(N_PAD, np.float32)
    dinv_pad[:N_NODES] = dinv

    c_arr = dst // PER_CORE
    b_arr = (dst % PER_CORE) // 128
    slot = dst % 128
    q_arr = src // CHUNK
    loc = (src % CHUNK).astype(np.int64)

    counts = np.zeros((NCORES, NB, NCHUNK), np.int64)
    np.add.at(counts, (c_arr, b_arr, q_arr), 1)
    Q = int(-(-counts.max() // 128))          # flat group quota per cell
    CELL = Q * 128
    NG = NB * Q                                # groups per chunk stream

    # rank of each edge within its cell
    cell_id = (c_arr * NB + b_arr) * NCHUNK + q_arr
    order = np.argsort(cell_id, kind='stable')
    cs = counts.reshape(-1)
    starts = np.zeros_like(cs)
    starts[1:] = np.cumsum(cs)[:-1]
    rank = np.arange(E) - np.repeat(starts, cs)

    loc_pad = np.zeros(NCORES * NB * NCHUNK * CELL, np.int16)
    slot_pad = np.full(NCORES * NB * NCHUNK * CELL, -1.0, np.float32)
    pos = cell_id[order] * CELL + rank
    loc_pad[pos] = loc[order].astype(np.int16)
    slot_pad[pos] = slot[order].astype(np.float32)
    loc_pad = loc_pad.reshape(NCORES, NB, NCHUNK, CELL)
    slot_pad = slot_pad.reshape(NCORES, NB, NCHUNK, CELL)

    y_node = np.asarray(y)[np.asarray(batch)]           # [N, Y_DIM]
    h0 = np.concatenate([np.asarray(x), y_node], axis=1).astype(np.float32)
    h0_pad = np.zeros((N_PAD, IN_DIM), np.float32)
    h0_pad[:N_NODES] = h0

    iota = np.tile(np.arange(128, dtype=np.float32), (128, 1))
    ident = np.eye(128, dtype=np.float32)

    h0T_full = np.ascontiguousarray(h0_pad.T)                        # [6, N_PAD]
    dinv_col_full = np.ascontiguousarray(
        dinv_pad.reshape(N_PAD // 128, 128).T)                       # [128, 784]

    in_maps = []
    for c in range(NCORES):
        lo = c * PER_CORE
        sl = slice(lo, lo + PER_CORE)
        dv = dinv_pad[sl]
        m = dict(
            h0T=np.ascontiguousarray(h0_pad[sl].T),                  # [6, 12544]
            h0T_full=h0T_full, dinv_col_full=dinv_col_full,
            dinv_col=np.ascontiguousarray(dv.reshape(NB, 128).T),    # [128, NB]
            dinv_bcast=np.tile(dv, (128, 1)),                        # [128, 12544]
            iota=iota, ident=ident,
        )
        # dstloc: [128, NCHUNK, NG]  (partition = edge pos in group)
        dl = slot_pad[c].transpose(1, 0, 2).reshape(NCHUNK, NG, 128)
        m['dstloc'] = np.ascontiguousarray(dl.transpose(2, 0, 1))
        # idx streams packed [16, NG*8] -> replicate to [128, NG*8]
        for q in range(NCHUNK):
            st = loc_pad[c, :, q, :].reshape(-1)                     # [NG*128]
            packed = np.ascontiguousarray(
                st.reshape(NG * 8, 16).T)                            # [16, NG*8]
            m[f'idx{q}'] = np.tile(packed, (8, 1))
        in_maps.append(m)
    return Q, NG, in_maps


# --------------------------------------------------------------- bass build

_CACHE = {}


def _build(Q, NG):
    key = (Q, NG)
    if key in _CACHE:
        return _CACHE[key]
    import concourse.bass as bass
    import concourse.bacc as bacc
    import concourse.mybir as mybir
    import concourse.tile as tile
    from concourse.alu_op_type import AluOpType

    f32 = mybir.dt.float32
    i16 = mybir.dt.int16
    RELU = mybir.ActivationFunctionType.Relu
    IDENT = mybir.ActivationFunctionType.Identity

    nc = bacc.Bacc("TRN2", target_bir_lowering=False, debug=False,
                   enable_asserts=False, num_devices=NCORES,
                   num_swdge_queues=4)

    NBF = N_PAD // 128                  # 784 global blocks
    ins = {}
    for name, shape, dt in [
        ('h0T', [IN_DIM, PER_CORE], f32), ('dinv_col', [128, NB], f32),
        ('h0T_full', [IN_DIM, N_PAD], f32), ('dinv_col_full', [128, NBF], f32),
        ('dinv_bcast', [128, PER_CORE], f32), ('iota', [128, 128], f32),
        ('ident', [128, 128], f32), ('dstloc', [128, NCHUNK, NG], f32),
        ('W1', [IN_DIM, HID], f32), ('W2', [HID, HID], f32),
        ('W3', [HID, HID], f32), ('W4', [HID, 8], f32),
        ('b1c', [128, 1], f32), ('b2c', [128, 1], f32),
        ('b3c', [128, 1], f32), ('b4c', [8, 1], f32),
    ] + [(f'idx{q}', [128, NG * 8], i16) for q in range(NCHUNK)]:
        ins[name] = nc.dram_tensor(name, shape, dt, kind="ExternalInput").ap()

    st_out = nc.dram_tensor("st", [8, PER_CORE], f32, kind="ExternalOutput").ap()

    ncalls = -(-NG // GPC)

    with tile.TileContext(nc) as tc:
        with (
            tc.tile_pool(name="const", bufs=1) as cpool,
            tc.tile_pool(name="msg", bufs=8) as mpool,
            tc.tile_pool(name="ind", bufs=4) as ipool,
            tc.tile_pool(name="idxp", bufs=8) as xpool,
            tc.tile_pool(name="tb", bufs=3) as tpool,
            tc.tile_pool(name="acc", bufs=2, space="PSUM") as apool,
            tc.tile_pool(name="pt", bufs=2, space="PSUM") as ppool,
            tc.tile_pool(name="ph", bufs=2, space="PSUM") as hpool,
            tc.tile_pool(name="dram", bufs=1, space="DRAM") as dpool,
        ):
            # ---- resident constants (h0T / dinv_bcast streamed per block)
            C = {}
            for name in ['dinv_col', 'iota', 'ident',
                         'dstloc', 'W1', 'W2', 'W3', 'W4',
                         'b1c', 'b2c', 'b3c', 'b4c']:
                ap = ins[name]
                t = cpool.tile(list(ap.shape), ap.dtype, name=f'c_{name}')
                nc.sync.dma_start(t[:], ap)
                C[name] = t

            ag_in = [dpool.tile([PER_CORE, HID], f32, name=f'ag_in{l}')
                     for l in range(2)]
            ag_out = [dpool.tile([N_PAD, HID], f32, addr_space="Shared",
                                 name=f'ag_out{l}') for l in range(2)]

            # ---- layer-1 dense transform: T1'[n] = (h0[n] @ W1) * dinv[n]
            for b in range(NB):
                h0b = tpool.tile([IN_DIM, 128], f32, name='h0b')
                nc.sync.dma_start(h0b[:], ins['h0T'][:, b * 128:(b + 1) * 128])
                pt = ppool.tile([128, HID], f32, name='pt1', tag='pt')
                nc.tensor.matmul(pt[:], h0b[:],
                                 C['W1'][:], start=True, stop=True)
                t1 = tpool.tile([128, HID], f32, name='t1')
                nc.scalar.mul(t1[:], pt[:], C['dinv_col'][:, b:b + 1])
                nc.sync.dma_start(ag_in[0][b * 128:(b + 1) * 128, :], t1[:])

            nc.gpsimd.collective_compute(
                "AllGather", mybir.AluOpType.bypass,
                replica_groups=[list(range(NCORES))],
                ins=[ag_in[0][:].opt()], outs=[ag_out[0][:].opt()])

            def aggregate_layer(layer, table, self_src, bias, out_cb):
                """One GCN aggregation pass over all blocks.

                table: DRAM AP of the full gathered table [N_PAD, HID]
                self_src: DRAM AP with this core's own T' rows [PER_CORE, HID]
                out_cb(b, hT_tile): consumes the feature-major block result
                """
                msg_t = [None] * NCHUNK      # current gather tile per stream
                ind_t = [None] * NCHUNK
                for b in range(NB):
                    selfb = tpool.tile([128, HID], f32, name=f'self{layer}')
                    nc.sync.dma_start(
                        selfb[:], self_src[b * 128:(b + 1) * 128, :])
                    acc = apool.tile([128, 128], f32, name=f'acc{layer}', tag='acc')
                    nc.tensor.matmul(acc[:], selfb[:], C['ident'][:],
                                     start=True, stop=False)
                    for q in range(NCHUNK):
                        for k in range(Q):
                            g = b * Q + k
                            if g % GPC == 0:
                                n = min(GPC, NG - g)
                                xt = xpool.tile([128, n * 8], i16, name='xt')
                                nc.sync.dma_start(
                                    xt[:], ins[f'idx{q}'][:, g * 8:(g + n) * 8])
                                mt = mpool.tile([128, n, HID], f32, name='mt')
                                nc.gpsimd.dma_gather(
                                    mt[:], table[q * CHUNK:(q + 1) * CHUNK, :],
                                    xt[:], num_idxs=n * 128,
                                    num_idxs_reg=n * 128, elem_size=HID,
                                    single_packet=SINGLE_PACKET,
                                    queue_num=q)
                                msg_t[q] = (mt, g)
                            if g % IB == 0:
                                n = min(IB, NG - g)
                                it = ipool.tile([128, n, 128], f32, name='it')
                                dl = C['dstloc'][:, q, g:g + n]
                                nc.vector.tensor_tensor(
                                    it[:], dl.to_broadcast([128, n, 128]),
                                    C['iota'][:].unsqueeze(1)
                                        .to_broadcast([128, n, 128]),
                                    op=AluOpType.is_equal)
                                ind_t[q] = (it, g)
                            mt, mg = msg_t[q]
                            it, ig = ind_t[q]
                            last = (q == NCHUNK - 1) and (k == Q - 1)
                            nc.tensor.matmul(acc[:], mt[:, g - mg, :],
                                             it[:, g - ig, :],
                                             start=False, stop=last)
                    # epilogue: hT = relu(acc * dinv_bcast + bias)
                    dvb = tpool.tile([128, 128], f32, name=f'dvb{layer}')
                    nc.sync.dma_start(
                        dvb[:], ins['dinv_bcast'][:, b * 128:(b + 1) * 128])
                    sT = tpool.tile([128, 128], f32, name=f'sT{layer}')
                    nc.vector.tensor_tensor(
                        sT[:], acc[:], dvb[:], op=AluOpType.mult)
                    hT = tpool.tile([128, 128], f32, name=f'hT{layer}')
                    nc.scalar.activation(hT[:], sT[:], RELU, bias=bias[:])
                    out_cb(b, hT)

            # ---- layer 1 aggregation; build T2 table per block
            def l1_out(b, hT):
                pt = ppool.tile([128, HID], f32, name='pt2', tag='pt')
                nc.tensor.matmul(pt[:], hT[:], C['W2'][:], start=True, stop=True)
                t2 = tpool.tile([128, HID], f32, name='t2')
                nc.scalar.mul(t2[:], pt[:], C['dinv_col'][:, b:b + 1])
                nc.sync.dma_start(ag_in[1][b * 128:(b + 1) * 128, :], t2[:])

            aggregate_layer(0, ag_out[0][:], ag_in[0][:], C['b1c'], l1_out)

            nc.gpsimd.collective_compute(
                "AllGather", mybir.AluOpType.bypass,
                replica_groups=[list(range(NCORES))],
                ins=[ag_in[1][:].opt()], outs=[ag_out[1][:].opt()])

            # ---- layer 2 aggregation; head per block
            def l2_out(b, hT):
                pm = hpool.tile([128, 128], f32, name='pm', tag='ph')
                nc.tensor.matmul(pm[:], C['W3'][:], hT[:], start=True, stop=True)
                m1 = tpool.tile([128, 128], f32, name='m1')
                nc.scalar.activation(m1[:], pm[:], RELU, bias=C['b3c'][:])
                ps = hpool.tile([8, 128], f32, name='ps', tag='ph')
                nc.tensor.matmul(ps[:], C['W4'][:], m1[:], start=True, stop=True)
                so = tpool.tile([8, 128], f32, name='so')
                nc.scalar.activation(so[:], ps[:], IDENT, bias=C['b4c'][:])
                nc.sync.dma_start(st_out[:, b * 128:(b + 1) * 128], so[:])

            aggregate_layer(1, ag_out[1][:], ag_in[1][:], C['b2c'], l2_out)

    nc.compile()
    _CACHE[key] = nc
    return nc


# ------------------------------------------------------------------- driver

def _install_profile_hook():
    """Wire antenv.axon_hooks -> ctypes NTFF profile against libaxon_pjrt.so
    (the agent image ships the .so but not the antenv glue)."""
    import sys, types, ctypes, contextlib
    try:
        from antenv.axon_hooks import get_axon_ntff_profile_hook  # noqa
        return True
    except ImportError:
        pass
    so_path = '/opt/axon/libaxon_pjrt.so'
    try:
        lib = ctypes.CDLL(so_path)
    except OSError:
        return False
    if not hasattr(lib, 'axon_start_nrt_profile'):
        return False
    lib.axon_start_nrt_profile.argtypes = [ctypes.POINTER(ctypes.c_int64),
                                           ctypes.c_size_t]
    lib.axon_start_nrt_profile.restype = ctypes.c_int64
    lib.axon_stop_nrt_profile.argtypes = [ctypes.c_char_p]
    lib.axon_stop_nrt_profile.restype = ctypes.c_int64

    @contextlib.contextmanager
    def _hook(output_dir, device_ids):
        import jax
        jax.devices()
        if device_ids:
            ids = (ctypes.c_int64 * len(device_ids))(*device_ids)
            rc = lib.axon_start_nrt_profile(ids, len(device_ids))
        else:
            rc = lib.axon_start_nrt_profile(None, 0)
        if rc != 0:
            raise RuntimeError(f"axon_start_nrt_profile rc={rc}")
        try:
            yield
        finally:
            n = lib.axon_stop_nrt_profile(str(output_dir).encode())
            print(f"profile: {n} ntff file(s) written to {output_dir}")

    mod = types.ModuleType('antenv.axon_hooks')
    _h = [_hook]
    mod.set_axon_ntff_profile_hook = lambda h: _h.__setitem__(0, h)
    mod.get_axon_ntff_profile_hook = lambda: _h[0]
    sys.modules['antenv.axon_hooks'] = mod
    import antenv
    antenv.axon_hooks = mod
    return True


def kernel(x, edge_index, y, batch, W1, b1, W2, b2, W3, b3, W4, b4,
           _trace=False, _tmpdir=None):
    from concourse.bass_utils import run_bass_kernel_spmd

    if _trace:
        _trace = _install_profile_hook()
        if _trace:
            import concourse.bass_utils as _bu
            _bu.upload_artifacts = lambda d: f"local://{d}"

    Q, NG, in_maps = _prep(x, edge_index, y, batch)
    consts = dict(
        W1=np.asarray(W1, np.float32), W2=np.asarray(W2, np.float32),
        W3=np.asarray(W3, np.float32), W4=np.asarray(W4, np.float32),
        b1c=np.asarray(b1, np.float32).reshape(128, 1),
        b2c=np.asarray(b2, np.float32).reshape(128, 1),
        b3c=np.asarray(b3, np.float32).reshape(128, 1),
        b4c=np.asarray(b4, np.float32).reshape(8, 1),
    )
    for m in in_maps:
        m.update(consts)

    nc = _build(Q, NG)
    res = run_bass_kernel_spmd(nc, in_maps, core_ids=list(range(NCORES)),
                               trace=_trace, tmpdir=_tmpdir)
    st = np.concatenate([res.results[c]['st'] for c in range(NCORES)], axis=1)
    st = st[:, :N_NODES]
    s = np.ascontiguousarray(st[:X_DIM].T)
    t = np.ascontiguousarray(st[X_DIM:].T)
    if _trace:
        kernel._last_results = res
    return (s, t)



# revision 2
# speedup vs baseline: 2.0507x; 2.0507x over previous
"""Trainium2 Bass kernel for ConditioningGNN (2-layer GCN + MLP head).

Math (per reference):
  src,dst,norm = gcn_norm(edge_index)  with self-loops, norm = dinv[src]*dinv[dst]
  h0 = [x | y[batch]]
  h1 = relu(A' @ (h0 W1) + b1);  h2 = relu(A' @ (h1 W2) + b2)
  st = relu(h2 W3 + b3) W4 + b4;  return split(st)

Device decomposition (8 NeuronCores, one SPMD program):
  * Nodes padded to 102400 = 8*12800; core c owns dst rows [c*12800,(c+1)*12800),
    NB=100 blocks of 128 dst slots each.
  * Layer 1 does NO device gather: A'(h0 W1) = (A' hs) W1 with hs = h0*dinv.
    The per-edge 6-dim hs[src] values (self-loops appended as ordinary edges)
    are pre-gathered on host into bf16 streams grouped by dst block; on device
    P[6,slot] accumulates hsT_grp.T @ onehot_grp on PE, then one W1^T @ P
    matmul per block. Epilogue scales by dinv[dst], relu(+b1) -> h1 block,
    then t2 = (h1 W2)*dinv rows (bf16) stream to the layer-2 table.
  * The t2 table is AllGathered in 4 quarter collectives (quarter q of every
    core's slice -> shared chunk buffer q), so chunk-q edge gathers can start
    as soon as quarter q has landed.
  * Layer 2 aggregation is the baseline scheme in bf16: int16 dma_gather of
    256B t2 rows per edge from chunk q, PSUM accumulates msg.T @ onehot per
    128-edge group; self-loop via one lhsT=own-rows, rhs=I matmul.
  * Epilogue per block: *dinv (DVE) -> relu(+b2) (ACT) -> head W3/W4 on PE.
"""
import sys
sys.path.insert(0, '/opt/trn_rl_repo')
sys.path.insert(0, '/opt/pypackages')

import numpy as np
import ml_dtypes

BF16 = ml_dtypes.bfloat16

N_NODES = 100000
NCORES = 8
NB = 100                            # dst blocks per core
PER_CORE = NB * 128                 # 12800
N_PAD = NCORES * PER_CORE           # 102400
NQ = 4                              # src chunks (table quarters)
QROWS = PER_CORE // NQ              # 3200 rows each core contributes per chunk
CHUNK = NCORES * QROWS              # 25600 rows per gathered chunk
HID = 128
X_DIM = 4
Y_DIM = 2
IN_DIM = X_DIM + Y_DIM
GPC = 20                            # L2 groups per gather call
IB = 8                              # L2 groups per indicator build
SINGLE_PACKET = False               # >64 descriptors/engine per packet hangs HW


# ----------------------------------------------------------------- host prep

def _rank_in_cell(cell_id, n_cells):
    order = np.argsort(cell_id, kind='stable')
    cs = np.bincount(cell_id, minlength=n_cells)
    starts = np.zeros_like(cs)
    starts[1:] = np.cumsum(cs)[:-1]
    rank = np.arange(cell_id.shape[0], dtype=np.int64) - np.repeat(starts, cs)
    return order, rank


def _prep(x, edge_index, y, batch):
    """Bucket edges, build per-core device arrays. Returns (G, Q, in_maps)."""
    src = np.asarray(edge_index[0], dtype=np.int64)
    dst = np.asarray(edge_index[1], dtype=np.int64)
    E = src.shape[0]

    deg = np.bincount(dst, minlength=N_NODES).astype(np.float64) + 1.0
    dinv = (1.0 / np.sqrt(deg)).astype(np.float32)
    dinv_pad = np.zeros(N_PAD, np.float32)
    dinv_pad[:N_NODES] = dinv

    y_node = np.asarray(y)[np.asarray(batch)]                # [N, Y_DIM]
    h0 = np.concatenate([np.asarray(x), y_node], axis=1).astype(np.float32)
    hs = h0 * dinv[:, None]                                  # [N, 6]

    # ---- L1 streams: edges + self loops, bucketed by (dst core, dst block)
    loop = np.arange(N_NODES, dtype=np.int64)
    src1 = np.concatenate([src, loop])
    dst1 = np.concatenate([dst, loop])
    c1 = dst1 // PER_CORE
    blk1 = (dst1 % PER_CORE) // 128
    slot1 = (dst1 % 128).astype(np.float32)
    cnt1 = np.bincount(c1 * NB + blk1, minlength=NCORES * NB)
    G = int(-(-cnt1.max() // 128))
    CAP1 = G * 128
    cell1 = c1 * NB + blk1
    order1, rank1 = _rank_in_cell(cell1, NCORES * NB)
    pos1 = cell1[order1] * CAP1 + rank1
    hsv = np.zeros((NCORES * NB * CAP1, IN_DIM), np.float32)
    sl1 = np.full(NCORES * NB * CAP1, -1.0, np.float32)
    hsv[pos1] = hs[src1[order1]]
    sl1[pos1] = slot1[order1]
    hsv = hsv.reshape(NCORES, NB * G, 128, IN_DIM)
    sl1 = sl1.reshape(NCORES, NB * G, 128)

    # ---- L2 gather streams: original edges by (dst core, dst block, src q)
    c2 = dst // PER_CORE
    blk2 = (dst % PER_CORE) // 128
    slot2 = (dst % 128).astype(np.float32)
    q2 = (src % PER_CORE) // QROWS
    loc2 = (src // PER_CORE) * QROWS + (src % PER_CORE) % QROWS   # < 25600
    cnt2 = np.bincount((c2 * NB + blk2) * NQ + q2,
                       minlength=NCORES * NB * NQ)
    Q = int(-(-cnt2.max() // 128))
    CELL2 = Q * 128
    NG2 = NB * Q
    cell2 = (c2 * NB + blk2) * NQ + q2
    order2, rank2 = _rank_in_cell(cell2, NCORES * NB * NQ)
    pos2 = cell2[order2] * CELL2 + rank2
    loc_pad = np.zeros(NCORES * NB * NQ * CELL2, np.int16)
    slot_pad = np.full(NCORES * NB * NQ * CELL2, -1.0, np.float32)
    loc_pad[pos2] = loc2[order2].astype(np.int16)
    slot_pad[pos2] = slot2[order2]
    loc_pad = loc_pad.reshape(NCORES, NB, NQ, CELL2)
    slot_pad = slot_pad.reshape(NCORES, NB, NQ, CELL2)

    iota = np.tile(np.arange(128, dtype=np.float32), (128, 1)).astype(BF16)
    ident = np.eye(128, dtype=np.float32).astype(BF16)

    in_maps = []
    for c in range(NCORES):
        lo = c * PER_CORE
        dv = dinv_pad[lo:lo + PER_CORE]
        m = dict(
            hsT=np.ascontiguousarray(
                hsv[c].transpose(1, 0, 2).reshape(128, NB * G * IN_DIM)
            ).astype(BF16),                                       # [128, NB*G*6]
            dstloc1=np.ascontiguousarray(sl1[c].T).astype(BF16),  # [128, NB*G]
            dinv_col=np.ascontiguousarray(dv.reshape(NB, 128).T), # [128, NB]
            dinv_bcast=np.tile(dv, (128, 1)),                     # [128, 12800]
            iota=iota, ident=ident,
        )
        # dstloc2: [128, NQ, NG2]  (partition = edge pos in group)
        dl = slot_pad[c].transpose(1, 0, 2).reshape(NQ, NG2, 128)
        m['dstloc2'] = np.ascontiguousarray(dl.transpose(2, 0, 1)).astype(BF16)
        # idx streams packed [16, NG2*8] -> replicate to [128, NG2*8]
        for q in range(NQ):
            st = loc_pad[c, :, q, :].reshape(-1)                  # [NG2*128]
            packed = np.ascontiguousarray(
                st.reshape(NG2 * 8, 16).T)                        # [16, NG2*8]
            m[f'idx{q}'] = np.tile(packed, (8, 1))
        in_maps.append(m)
    return G, Q, in_maps


# --------------------------------------------------------------- bass build

_CACHE = {}


def _build(G, Q):
    key = (G, Q)
    if key in _CACHE:
        return _CACHE[key]
    import concourse.bass as bass
    import concourse.bacc as bacc
    import concourse.mybir as mybir
    import concourse.tile as tile
    from concourse.alu_op_type import AluOpType

    f32 = mybir.dt.float32
    bf16 = mybir.dt.bfloat16
    i16 = mybir.dt.int16
    RELU = mybir.ActivationFunctionType.Relu
    IDENT = mybir.ActivationFunctionType.Identity

    NG2 = NB * Q
    IBL1 = -(-G // 2)                   # L1 indicator builds: 2 per block

    nc = bacc.Bacc("TRN2", target_bir_lowering=False, debug=False,
                   enable_asserts=False, num_devices=NCORES,
                   num_swdge_queues=4)

    ins = {}
    for name, shape, dt in [
        ('hsT', [128, NB * G * IN_DIM], bf16),
        ('dstloc1', [128, NB * G], bf16),
        ('dstloc2', [128, NQ, NG2], bf16),
        ('dinv_col', [128, NB], f32), ('dinv_bcast', [128, PER_CORE], f32),
        ('iota', [128, 128], bf16), ('ident', [128, 128], bf16),
        ('W1b', [IN_DIM, HID], bf16), ('W2b', [HID, HID], bf16),
        ('W3b', [HID, HID], bf16), ('W4b', [HID, 8], bf16),
        ('b1c', [128, 1], f32), ('b2c', [128, 1], f32),
        ('b3c', [128, 1], f32), ('b4c', [8, 1], f32),
    ] + [(f'idx{q}', [128, NG2 * 8], i16) for q in range(NQ)]:
        ins[name] = nc.dram_tensor(name, shape, dt, kind="ExternalInput").ap()

    st_out = nc.dram_tensor("st", [8, PER_CORE], f32, kind="ExternalOutput").ap()

    with tile.TileContext(nc) as tc:
        with (
            tc.tile_pool(name="const", bufs=1) as cpool,
            tc.tile_pool(name="msg", bufs=6) as mpool,
            tc.tile_pool(name="ind", bufs=4) as ipool,
            tc.tile_pool(name="ind1", bufs=4) as i1pool,
            tc.tile_pool(name="tb", bufs=10) as tpool,
            tc.tile_pool(name="p6", bufs=2, space="PSUM") as p6pool,
            tc.tile_pool(name="pp", bufs=2, space="PSUM") as ppool,
            tc.tile_pool(name="acc", bufs=2, space="PSUM") as apool,
            tc.tile_pool(name="ph", bufs=2, space="PSUM") as hpool,
            tc.tile_pool(name="dram", bufs=1, space="DRAM") as dpool,
        ):
            # ---- resident constants (everything but the gather tables)
            C = {}
            for name in ins:
                ap = ins[name]
                t = cpool.tile(list(ap.shape), ap.dtype, name=f'c_{name}')
                nc.sync.dma_start(t[:], ap)
                C[name] = t

            ag_in = dpool.tile([PER_CORE, HID], bf16, name='ag_in')
            ag_out = [dpool.tile([CHUNK, HID], bf16, addr_space="Shared",
                                 name=f'ag_out{q}') for q in range(NQ)]

            # ---- layer 1: stream host-gathered hs[src] values, no gather
            for b in range(NB):
                P = p6pool.tile([IN_DIM, 128], f32, name='P', tag='P')
                it1 = None
                for j in range(G):
                    g = b * G + j
                    if j % IBL1 == 0:
                        n = min(IBL1, G - j)
                        it1 = i1pool.tile([128, n, 128], bf16, name='it1')
                        dl = C['dstloc1'][:, g:g + n]
                        nc.vector.tensor_tensor(
                            it1[:], dl.to_broadcast([128, n, 128]),
                            C['iota'][:].unsqueeze(1)
                                .to_broadcast([128, n, 128]),
                            op=AluOpType.is_equal)
                        i1g = j
                    nc.tensor.matmul(
                        P[:], C['hsT'][:, g * IN_DIM:(g + 1) * IN_DIM],
                        it1[:, j - i1g, :],
                        start=(j == 0), stop=(j == G - 1))
                Pb = tpool.tile([IN_DIM, 128], bf16, name='Pb')
                nc.scalar.copy(Pb[:], P[:])
                acc1 = ppool.tile([128, 128], f32, name='acc1', tag='pp')
                nc.tensor.matmul(acc1[:], C['W1b'][:], Pb[:],
                                 start=True, stop=True)
                sT = tpool.tile([128, 128], f32, name='sT1')
                nc.vector.tensor_tensor(
                    sT[:], acc1[:],
                    C['dinv_bcast'][:, b * 128:(b + 1) * 128],
                    op=AluOpType.mult)
                hT = tpool.tile([128, 128], bf16, name='hT1')
                nc.scalar.activation(hT[:], sT[:], RELU, bias=C['b1c'][:])
                pt2 = ppool.tile([128, 128], f32, name='pt2', tag='pp')
                nc.tensor.matmul(pt2[:], hT[:], C['W2b'][:],
                                 start=True, stop=True)
                t2 = tpool.tile([128, 128], bf16, name='t2')
                nc.scalar.mul(t2[:], pt2[:], C['dinv_col'][:, b:b + 1])
                nc.sync.dma_start(ag_in[b * 128:(b + 1) * 128, :], t2[:])

            # ---- table quarters -> shared chunks (start as quarters finish)
            for q in range(NQ):
                nc.gpsimd.collective_compute(
                    "AllGather", mybir.AluOpType.bypass,
                    replica_groups=[list(range(NCORES))],
                    ins=[ag_in[q * QROWS:(q + 1) * QROWS, :].opt()],
                    outs=[ag_out[q][:].opt()])

            # ---- layer 2 aggregation + head
            msg_t = [None] * NQ
            ind_t = [None] * NQ
            for b in range(NB):
                selfb = tpool.tile([128, HID], bf16, name='selfb')
                nc.sync.dma_start(selfb[:], ag_in[b * 128:(b + 1) * 128, :])
                acc = apool.tile([128, 128], f32, name='acc', tag='acc')
                nc.tensor.matmul(acc[:], selfb[:], C['ident'][:],
                                 start=True, stop=False)
                for q in range(NQ):
                    for k in range(Q):
                        g = b * Q + k
                        if g % GPC == 0:
                            n = min(GPC, NG2 - g)
                            mt = mpool.tile([128, n, HID], bf16, name='mt')
                            nc.gpsimd.dma_gather(
                                mt[:], ag_out[q][:],
                                C[f'idx{q}'][:, g * 8:(g + n) * 8],
                                num_idxs=n * 128,
                                num_idxs_reg=n * 128, elem_size=HID,
                                single_packet=SINGLE_PACKET,
                                queue_num=q)
                            msg_t[q] = (mt, g)
                        if g % IB == 0:
                            n = min(IB, NG2 - g)
                            it = ipool.tile([128, n, 128], bf16, name='it')
                            dl = C['dstloc2'][:, q, g:g + n]
                            nc.vector.tensor_tensor(
                                it[:], dl.to_broadcast([128, n, 128]),
                                C['iota'][:].unsqueeze(1)
                                    .to_broadcast([128, n, 128]),
                                op=AluOpType.is_equal)
                            ind_t[q] = (it, g)
                        mt, mg = msg_t[q]
                        it, ig = ind_t[q]
                        last = (q == NQ - 1) and (k == Q - 1)
                        nc.tensor.matmul(acc[:], mt[:, g - mg, :],
                                         it[:, g - ig, :],
                                         start=False, stop=last)
                # epilogue: h2 = relu(acc * dinv + b2); head W3/W4
                sT2 = tpool.tile([128, 128], f32, name='sT2')
                nc.vector.tensor_tensor(
                    sT2[:], acc[:],
                    C['dinv_bcast'][:, b * 128:(b + 1) * 128],
                    op=AluOpType.mult)
                hT2 = tpool.tile([128, 128], bf16, name='hT2')
                nc.scalar.activation(hT2[:], sT2[:], RELU, bias=C['b2c'][:])
                pm = hpool.tile([128, 128], f32, name='pm', tag='ph')
                nc.tensor.matmul(pm[:], C['W3b'][:], hT2[:],
                                 start=True, stop=True)
                m1 = tpool.tile([128, 128], bf16, name='m1')
                nc.scalar.activation(m1[:], pm[:], RELU, bias=C['b3c'][:])
                ps = hpool.tile([8, 128], f32, name='ps', tag='ph')
                nc.tensor.matmul(ps[:], C['W4b'][:], m1[:],
                                 start=True, stop=True)
                so = tpool.tile([8, 128], f32, name='so')
                nc.scalar.activation(so[:], ps[:], IDENT, bias=C['b4c'][:])
                nc.sync.dma_start(st_out[:, b * 128:(b + 1) * 128], so[:])

    nc.compile()
    _CACHE[key] = nc
    return nc


# ------------------------------------------------------------------- driver

def _install_profile_hook():
    """Wire antenv.axon_hooks -> ctypes NTFF profile against libaxon_pjrt.so
    (the agent image ships the .so but not the antenv glue)."""
    import sys, types, ctypes, contextlib
    try:
        from antenv.axon_hooks import get_axon_ntff_profile_hook  # noqa
        return True
    except ImportError:
        pass
    so_path = '/opt/axon/libaxon_pjrt.so'
    try:
        lib = ctypes.CDLL(so_path)
    except OSError:
        return False
    if not hasattr(lib, 'axon_start_nrt_profile'):
        return False
    lib.axon_start_nrt_profile.argtypes = [ctypes.POINTER(ctypes.c_int64),
                                           ctypes.c_size_t]
    lib.axon_start_nrt_profile.restype = ctypes.c_int64
    lib.axon_stop_nrt_profile.argtypes = [ctypes.c_char_p]
    lib.axon_stop_nrt_profile.restype = ctypes.c_int64

    @contextlib.contextmanager
    def _hook(output_dir, device_ids):
        import jax
        jax.devices()
        if device_ids:
            ids = (ctypes.c_int64 * len(device_ids))(*device_ids)
            rc = lib.axon_start_nrt_profile(ids, len(device_ids))
        else:
            rc = lib.axon_start_nrt_profile(None, 0)
        if rc != 0:
            raise RuntimeError(f"axon_start_nrt_profile rc={rc}")
        try:
            yield
        finally:
            n = lib.axon_stop_nrt_profile(str(output_dir).encode())
            print(f"profile: {n} ntff file(s) written to {output_dir}")

    mod = types.ModuleType('antenv.axon_hooks')
    _h = [_hook]
    mod.set_axon_ntff_profile_hook = lambda h: _h.__setitem__(0, h)
    mod.get_axon_ntff_profile_hook = lambda: _h[0]
    sys.modules['antenv.axon_hooks'] = mod
    import antenv
    antenv.axon_hooks = mod
    return True


def kernel(x, edge_index, y, batch, W1, b1, W2, b2, W3, b3, W4, b4,
           _trace=False, _tmpdir=None):
    from concourse.bass_utils import run_bass_kernel_spmd

    if _trace:
        _trace = _install_profile_hook()
        if _trace:
            import concourse.bass_utils as _bu
            _bu.upload_artifacts = lambda d: f"local://{d}"

    G, Q, in_maps = _prep(x, edge_index, y, batch)
    consts = dict(
        W1b=np.asarray(W1, np.float32).astype(BF16),
        W2b=np.asarray(W2, np.float32).astype(BF16),
        W3b=np.asarray(W3, np.float32).astype(BF16),
        W4b=np.asarray(W4, np.float32).astype(BF16),
        b1c=np.asarray(b1, np.float32).reshape(128, 1),
        b2c=np.asarray(b2, np.float32).reshape(128, 1),
        b3c=np.asarray(b3, np.float32).reshape(128, 1),
        b4c=np.asarray(b4, np.float32).reshape(8, 1),
    )
    for m in in_maps:
        m.update(consts)

    nc = _build(G, Q)
    res = run_bass_kernel_spmd(nc, in_maps, core_ids=list(range(NCORES)),
                               trace=_trace, tmpdir=_tmpdir)
    st = np.concatenate([res.results[c]['st'] for c in range(NCORES)], axis=1)
    st = st[:, :N_NODES]
    s = np.ascontiguousarray(st[:X_DIM].T)
    t = np.ascontiguousarray(st[X_DIM:].T)
    if _trace:
        kernel._last_results = res
    return (s, t)


# revision 4
# speedup vs baseline: 2.4580x; 1.1986x over previous
"""Trainium2 Bass kernel for ConditioningGNN (2-layer GCN + MLP head).

Math (per reference):
  src,dst,norm = gcn_norm(edge_index)  with self-loops, norm = dinv[src]*dinv[dst]
  h0 = [x | y[batch]]
  h1 = relu(A' @ (h0 W1) + b1);  h2 = relu(A' @ (h1 W2) + b2)
  st = relu(h2 W3 + b3) W4 + b4;  return split(st)

Device decomposition (8 NeuronCores, one SPMD program):
  * Nodes padded to 102400 = 8*12800; core c owns dst rows [c*12800,(c+1)*12800),
    NB=100 blocks of 128 dst slots each.
  * All onehot indicators are PRECOMPUTED ON HOST with the GCN edge norm
    folded into the indicator value (ind[e,slot] = norm_e), streamed as bf16.
    No DVE is_equal work and no epilogue dinv multiply remain on device.
  * Layer 1 does NO device gather: A'(h0 W1) = (A' n h0[src]) W1. Host
    pre-gathers the 6-dim h0[src] per edge (self-loops appended as ordinary
    edges with norm dinv^2); device accumulates P[6,slot] += msgs.T @ ind on
    PE per block, then one W1^T @ P matmul; relu(+b1) -> h1; t2 = (h1 W2)*dinv
    rows (bf16) go to the layer-2 table.
  * The t2 table is AllGathered in 4 quarter collectives (quarter q of every
    core's slice -> shared chunk buffer q), so chunk-q edge gathers can start
    as soon as quarter q has landed.
  * Layer 2: int16 dma_gather of 256B t2 rows per edge from chunk q (bf16,
    single-packet descriptors), PSUM accumulates msg.T @ ind per 128-edge
    group (ind value = dinv[dst]); self-loop via rhs=diag(dinv) matmul.
  * Epilogue per block: relu(acc+b2) straight from PSUM -> head W3/W4 on PE.
"""
import sys
sys.path.insert(0, '/opt/trn_rl_repo')
sys.path.insert(0, '/opt/pypackages')

import numpy as np
import ml_dtypes

BF16 = ml_dtypes.bfloat16

N_NODES = 100000
NCORES = 8
NB = 100                            # dst blocks per core
PER_CORE = NB * 128                 # 12800
N_PAD = NCORES * PER_CORE           # 102400
NQ = 4                              # src chunks (table quarters)
QROWS = PER_CORE // NQ              # 3200 rows each core contributes per chunk
CHUNK = NCORES * QROWS              # 25600 rows per gathered chunk
HID = 128
X_DIM = 4
Y_DIM = 2
IN_DIM = X_DIM + Y_DIM
GPC = 8                             # L2 groups per gather call (64 desc/engine)
IB = 8                              # L2 groups per indicator load
SINGLE_PACKET = True                # 64 descriptors/engine per packet is the max


# ----------------------------------------------------------------- host prep

def _rank_in_cell(cell_id, n_cells):
    order = np.argsort(cell_id, kind='stable')
    cs = np.bincount(cell_id, minlength=n_cells)
    starts = np.zeros_like(cs)
    starts[1:] = np.cumsum(cs)[:-1]
    rank = np.arange(cell_id.shape[0], dtype=np.int64) - np.repeat(starts, cs)
    return order, rank


def _prep(x, edge_index, y, batch):
    """Bucket edges, build per-core device arrays. Returns (G, Q, in_maps)."""
    src = np.asarray(edge_index[0], dtype=np.int64)
    dst = np.asarray(edge_index[1], dtype=np.int64)

    deg = np.bincount(dst, minlength=N_NODES).astype(np.float64) + 1.0
    dinv = (1.0 / np.sqrt(deg)).astype(np.float32)
    dinv_pad = np.zeros(N_PAD, np.float32)
    dinv_pad[:N_NODES] = dinv

    y_node = np.asarray(y)[np.asarray(batch)]                # [N, Y_DIM]
    h0 = np.concatenate([np.asarray(x), y_node], axis=1).astype(np.float32)

    # ---- L1 streams: edges + self loops, bucketed by (dst core, dst block)
    loop = np.arange(N_NODES, dtype=np.int64)
    src1 = np.concatenate([src, loop])
    dst1 = np.concatenate([dst, loop])
    norm1 = (dinv[src1] * dinv[dst1]).astype(np.float32)
    c1 = dst1 // PER_CORE
    blk1 = (dst1 % PER_CORE) // 128
    slot1 = (dst1 % 128).astype(np.int64)
    cell1 = c1 * NB + blk1
    cnt1 = np.bincount(cell1, minlength=NCORES * NB)
    G = int(-(-cnt1.max() // 128))
    CAP1 = G * 128
    order1, rank1 = _rank_in_cell(cell1, NCORES * NB)
    pos1 = cell1[order1] * CAP1 + rank1
    hsv = np.zeros((NCORES * NB * CAP1, IN_DIM), BF16)
    hsv[pos1] = h0[src1[order1]].astype(BF16)
    ind1 = np.zeros((NCORES * NB * CAP1, 128), BF16)
    ind1[pos1, slot1[order1]] = norm1[order1].astype(BF16)
    hsv = hsv.reshape(NCORES, NB * G, 128, IN_DIM)
    ind1 = ind1.reshape(NCORES, NB * G, 128, 128)

    # ---- L2 gather streams: original edges by (dst core, dst block, src q)
    c2 = dst // PER_CORE
    blk2 = (dst % PER_CORE) // 128
    slot2 = (dst % 128).astype(np.int64)
    q2 = (src % PER_CORE) // QROWS
    loc2 = (src // PER_CORE) * QROWS + (src % PER_CORE) % QROWS   # < 25600
    cell2 = (c2 * NB + blk2) * NQ + q2
    cnt2 = np.bincount(cell2, minlength=NCORES * NB * NQ)
    Q = int(-(-cnt2.max() // 128))
    CELL2 = Q * 128
    NG2 = NB * Q
    order2, rank2 = _rank_in_cell(cell2, NCORES * NB * NQ)
    pos2 = cell2[order2] * CELL2 + rank2
    loc_pad = np.zeros(NCORES * NB * NQ * CELL2, np.int16)
    loc_pad[pos2] = loc2[order2].astype(np.int16)
    ind2 = np.zeros((NCORES * NB * NQ * CELL2, 128), BF16)
    ind2[pos2, slot2[order2]] = dinv[dst[order2]].astype(BF16)
    loc_pad = loc_pad.reshape(NCORES, NB, NQ, CELL2)
    ind2 = ind2.reshape(NCORES, NB, NQ, Q, 128, 128)

    # identd: per-block diag(dinv) for the layer-2 self term
    ar = np.arange(128)
    identd = np.zeros((NCORES, NB, 128, 128), np.float32)
    dv_blocks = dinv_pad.reshape(NCORES, NB, 128)
    identd[:, :, ar, ar] = dv_blocks
    identd = np.ascontiguousarray(
        identd.transpose(0, 2, 1, 3)).reshape(NCORES, 128, NB * 128).astype(BF16)

    in_maps = []
    for c in range(NCORES):
        lo = c * PER_CORE
        dv = dinv_pad[lo:lo + PER_CORE]
        m = dict(
            hsT=np.ascontiguousarray(
                hsv[c].transpose(1, 0, 2).reshape(128, NB * G * IN_DIM)),
            ind1=np.ascontiguousarray(
                ind1[c].transpose(1, 0, 2).reshape(128, NB * G * 128)),
            ind2=np.ascontiguousarray(
                ind2[c].transpose(3, 1, 0, 2, 4).reshape(128, NQ * NG2 * 128)),
            identd=identd[c],
            dinv_col=np.ascontiguousarray(dv.reshape(NB, 128).T),  # [128, NB]
        )
        # idx streams packed [16, NG2*8] -> replicate to [128, NG2*8]
        for q in range(NQ):
            st = loc_pad[c, :, q, :].reshape(-1)                  # [NG2*128]
            packed = np.ascontiguousarray(
                st.reshape(NG2 * 8, 16).T)                        # [16, NG2*8]
            m[f'idx{q}'] = np.tile(packed, (8, 1))
        in_maps.append(m)
    return G, Q, in_maps


# --------------------------------------------------------------- bass build

_CACHE = {}


def _build(G, Q):
    key = (G, Q)
    if key in _CACHE:
        return _CACHE[key]
    import concourse.bass as bass
    import concourse.bacc as bacc
    import concourse.mybir as mybir
    import concourse.tile as tile

    f32 = mybir.dt.float32
    bf16 = mybir.dt.bfloat16
    i16 = mybir.dt.int16
    RELU = mybir.ActivationFunctionType.Relu
    IDENT = mybir.ActivationFunctionType.Identity

    NG2 = NB * Q

    nc = bacc.Bacc("TRN2", target_bir_lowering=False, debug=False,
                   enable_asserts=False, num_devices=NCORES,
                   num_swdge_queues=4)

    ins = {}
    for name, shape, dt in [
        ('hsT', [128, NB * G * IN_DIM], bf16),
        ('ind1', [128, NB * G * 128], bf16),
        ('ind2', [128, NQ * NG2 * 128], bf16),
        ('identd', [128, NB * 128], bf16),
        ('dinv_col', [128, NB], f32),
        ('W1b', [IN_DIM, HID], bf16), ('W2b', [HID, HID], bf16),
        ('W3b', [HID, HID], bf16), ('W4b', [HID, 8], bf16),
        ('b1c', [128, 1], f32), ('b2c', [128, 1], f32),
        ('b3c', [128, 1], f32), ('b4c', [8, 1], f32),
    ] + [(f'idx{q}', [128, NG2 * 8], i16) for q in range(NQ)]:
        ins[name] = nc.dram_tensor(name, shape, dt, kind="ExternalInput").ap()

    st_out = nc.dram_tensor("st", [8, PER_CORE], f32, kind="ExternalOutput").ap()

    with tile.TileContext(nc) as tc:
        with (
            tc.tile_pool(name="const", bufs=1) as cpool,
            tc.tile_pool(name="msg", bufs=12) as mpool,
            tc.tile_pool(name="ind", bufs=6) as ipool,
            tc.tile_pool(name="ind1", bufs=3) as i1pool,
            tc.tile_pool(name="tb", bufs=12) as tpool,
            tc.tile_pool(name="p6", bufs=2, space="PSUM") as p6pool,
            tc.tile_pool(name="pp", bufs=2, space="PSUM") as ppool,
            tc.tile_pool(name="acc", bufs=2, space="PSUM") as apool,
            tc.tile_pool(name="ph", bufs=2, space="PSUM") as hpool,
            tc.tile_pool(name="dram", bufs=1, space="DRAM") as dpool,
        ):
            # ---- resident constants (ind1/ind2 are streamed, not resident)
            C = {}
            for name in ins:
                if name in ('ind1', 'ind2'):
                    continue
                ap = ins[name]
                t = cpool.tile(list(ap.shape), ap.dtype, name=f'c_{name}')
                nc.sync.dma_start(t[:], ap)
                C[name] = t

            ag_in = dpool.tile([PER_CORE, HID], bf16, name='ag_in')
            ag_out = [dpool.tile([CHUNK, HID], bf16, addr_space="Shared",
                                 name=f'ag_out{q}') for q in range(NQ)]

            # ---- layer 1: stream host-gathered h0[src] + norm indicators
            for b in range(NB):
                it1 = i1pool.tile([128, G * 128], bf16, name='it1')
                nc.sync.dma_start(
                    it1[:], ins['ind1'][:, b * G * 128:(b + 1) * G * 128])
                P = p6pool.tile([IN_DIM, 128], f32, name='P', tag='P')
                for j in range(G):
                    g = b * G + j
                    nc.tensor.matmul(
                        P[:], C['hsT'][:, g * IN_DIM:(g + 1) * IN_DIM],
                        it1[:, j * 128:(j + 1) * 128],
                        start=(j == 0), stop=(j == G - 1))
                Pb = tpool.tile([IN_DIM, 128], bf16, name='Pb')
                nc.scalar.copy(Pb[:], P[:])
                acc1 = ppool.tile([128, 128], f32, name='acc1', tag='pp')
                nc.tensor.matmul(acc1[:], C['W1b'][:], Pb[:],
                                 start=True, stop=True)
                hT = tpool.tile([128, 128], bf16, name='hT1')
                nc.scalar.activation(hT[:], acc1[:], RELU, bias=C['b1c'][:])
                pt2 = ppool.tile([128, 128], f32, name='pt2', tag='pp')
                nc.tensor.matmul(pt2[:], hT[:], C['W2b'][:],
                                 start=True, stop=True)
                t2 = tpool.tile([128, 128], bf16, name='t2')
                nc.scalar.mul(t2[:], pt2[:], C['dinv_col'][:, b:b + 1])
                nc.sync.dma_start(ag_in[b * 128:(b + 1) * 128, :], t2[:])

            # ---- table quarters -> shared chunks (start as quarters finish)
            for q in range(NQ):
                nc.gpsimd.collective_compute(
                    "AllGather", mybir.AluOpType.bypass,
                    replica_groups=[list(range(NCORES))],
                    ins=[ag_in[q * QROWS:(q + 1) * QROWS, :].opt()],
                    outs=[ag_out[q][:].opt()])

            # ---- layer 2 aggregation + head
            msg_t = [None] * NQ
            ind_t = [None] * NQ
            for b in range(NB):
                selfb = tpool.tile([128, HID], bf16, name='selfb')
                nc.sync.dma_start(selfb[:], ag_in[b * 128:(b + 1) * 128, :])
                acc = apool.tile([128, 128], f32, name='acc', tag='acc')
                nc.tensor.matmul(acc[:], selfb[:],
                                 C['identd'][:, b * 128:(b + 1) * 128],
                                 start=True, stop=False)
                for q in range(NQ):
                    for k in range(Q):
                        g = b * Q + k
                        if g % GPC == 0:
                            n = min(GPC, NG2 - g)
                            mt = mpool.tile([128, n, HID], bf16, name='mt')
                            nc.gpsimd.dma_gather(
                                mt[:], ag_out[q][:],
                                C[f'idx{q}'][:, g * 8:(g + n) * 8],
                                num_idxs=n * 128,
                                num_idxs_reg=n * 128, elem_size=HID,
                                single_packet=SINGLE_PACKET,
                                queue_num=q)
                            msg_t[q] = (mt, g)
                        if g % IB == 0:
                            n = min(IB, NG2 - g)
                            it = ipool.tile([128, n * 128], bf16, name='it')
                            nc.sync.dma_start(
                                it[:],
                                ins['ind2'][:, (q * NG2 + g) * 128:
                                            (q * NG2 + g + n) * 128])
                            ind_t[q] = (it, g)
                        mt, mg = msg_t[q]
                        it, ig = ind_t[q]
                        last = (q == NQ - 1) and (k == Q - 1)
                        nc.tensor.matmul(
                            acc[:], mt[:, g - mg, :],
                            it[:, (g - ig) * 128:(g - ig + 1) * 128],
                            start=False, stop=last)
                # epilogue: h2 = relu(acc + b2); head W3/W4
                hT2 = tpool.tile([128, 128], bf16, name='hT2')
                nc.scalar.activation(hT2[:], acc[:], RELU, bias=C['b2c'][:])
                pm = hpool.tile([128, 128], f32, name='pm', tag='ph')
                nc.tensor.matmul(pm[:], C['W3b'][:], hT2[:],
                                 start=True, stop=True)
                m1 = tpool.tile([128, 128], bf16, name='m1')
                nc.scalar.activation(m1[:], pm[:], RELU, bias=C['b3c'][:])
                ps = hpool.tile([8, 128], f32, name='ps', tag='ph')
                nc.tensor.matmul(ps[:], C['W4b'][:], m1[:],
                                 start=True, stop=True)
                so = tpool.tile([8, 128], f32, name='so')
                nc.scalar.activation(so[:], ps[:], IDENT, bias=C['b4c'][:])
                nc.sync.dma_start(st_out[:, b * 128:(b + 1) * 128], so[:])

    nc.compile()
    _CACHE[key] = nc
    return nc


# ------------------------------------------------------------------- driver

def _install_profile_hook():
    """Wire antenv.axon_hooks -> ctypes NTFF profile against libaxon_pjrt.so
    (the agent image ships the .so but not the antenv glue)."""
    import sys, types, ctypes, contextlib
    try:
        from antenv.axon_hooks import get_axon_ntff_profile_hook  # noqa
        return True
    except ImportError:
        pass
    so_path = '/opt/axon/libaxon_pjrt.so'
    try:
        lib = ctypes.CDLL(so_path)
    except OSError:
        return False
    if not hasattr(lib, 'axon_start_nrt_profile'):
        return False
    lib.axon_start_nrt_profile.argtypes = [ctypes.POINTER(ctypes.c_int64),
                                           ctypes.c_size_t]
    lib.axon_start_nrt_profile.restype = ctypes.c_int64
    lib.axon_stop_nrt_profile.argtypes = [ctypes.c_char_p]
    lib.axon_stop_nrt_profile.restype = ctypes.c_int64

    @contextlib.contextmanager
    def _hook(output_dir, device_ids):
        import jax
        jax.devices()
        if device_ids:
            ids = (ctypes.c_int64 * len(device_ids))(*device_ids)
            rc = lib.axon_start_nrt_profile(ids, len(device_ids))
        else:
            rc = lib.axon_start_nrt_profile(None, 0)
        if rc != 0:
            raise RuntimeError(f"axon_start_nrt_profile rc={rc}")
        try:
            yield
        finally:
            n = lib.axon_stop_nrt_profile(str(output_dir).encode())
            print(f"profile: {n} ntff file(s) written to {output_dir}")

    mod = types.ModuleType('antenv.axon_hooks')
    _h = [_hook]
    mod.set_axon_ntff_profile_hook = lambda h: _h.__setitem__(0, h)
    mod.get_axon_ntff_profile_hook = lambda: _h[0]
    sys.modules['antenv.axon_hooks'] = mod
    import antenv
    antenv.axon_hooks = mod
    return True


def kernel(x, edge_index, y, batch, W1, b1, W2, b2, W3, b3, W4, b4,
           _trace=False, _tmpdir=None):
    from concourse.bass_utils import run_bass_kernel_spmd

    if _trace:
        _trace = _install_profile_hook()
        if _trace:
            import concourse.bass_utils as _bu
            _bu.upload_artifacts = lambda d: f"local://{d}"

    G, Q, in_maps = _prep(x, edge_index, y, batch)
    consts = dict(
        W1b=np.asarray(W1, np.float32).astype(BF16),
        W2b=np.asarray(W2, np.float32).astype(BF16),
        W3b=np.asarray(W3, np.float32).astype(BF16),
        W4b=np.asarray(W4, np.float32).astype(BF16),
        b1c=np.asarray(b1, np.float32).reshape(128, 1),
        b2c=np.asarray(b2, np.float32).reshape(128, 1),
        b3c=np.asarray(b3, np.float32).reshape(128, 1),
        b4c=np.asarray(b4, np.float32).reshape(8, 1),
    )
    for m in in_maps:
        m.update(consts)

    nc = _build(G, Q)
    res = run_bass_kernel_spmd(nc, in_maps, core_ids=list(range(NCORES)),
                               trace=_trace, tmpdir=_tmpdir)
    st = np.concatenate([res.results[c]['st'] for c in range(NCORES)], axis=1)
    st = st[:, :N_NODES]
    s = np.ascontiguousarray(st[:X_DIM].T)
    t = np.ascontiguousarray(st[X_DIM:].T)
    if _trace:
        kernel._last_results = res
    return (s, t)


# revision 5
# speedup vs baseline: 3.3907x; 1.3795x over previous
"""Trainium2 Bass kernel for ConditioningGNN (2-layer GCN + MLP head).

Math (per reference):
  src,dst,norm = gcn_norm(edge_index)  with self-loops, norm = dinv[src]*dinv[dst]
  h0 = [x | y[batch]]
  h1 = relu(A' @ (h0 W1) + b1);  h2 = relu(A' @ (h1 W2) + b2)
  st = relu(h2 W3 + b3) W4 + b4;  return split(st)

Device decomposition (8 NeuronCores, one SPMD program):
  * Nodes padded to 102400 = 8*12800; core c owns dst rows [c*12800,(c+1)*12800),
    NB=100 blocks of 128 dst slots each.
  * Layer 1 does NO device gather: A'(h0 W1) = (A' M) W1 where M[e] =
    norm_e * h0[src_e] is pre-gathered per edge ON HOST (self-loops appended
    as ordinary edges), streamed as bf16 grouped by dst block. Device builds
    onehot indicators on DVE (is_equal vs iota), accumulates P[6,slot] +=
    M_grp.T @ onehot on PE per block, then one W1^T @ P matmul; relu(+b1)
    -> h1; t2 = (h1 W2)*dinv rows (bf16) go to the layer-2 table.
  * The t2 table is AllGathered in 4 quarter collectives (quarter q of every
    core's slice -> shared chunk buffer q), so chunk-q edge gathers can start
    as soon as quarter q has landed.
  * Layer 2: int16 dma_gather of 256B t2 rows per edge from chunk q (bf16,
    single-packet, 64 descriptors/engine), PSUM accumulates msg.T @ onehot
    per 128-edge group; self-loop via rhs=I matmul; epilogue scales by
    dinv[dst] (DVE), relu(+b2) (ACT), head W3/W4 on PE.
"""
import sys
sys.path.insert(0, '/opt/trn_rl_repo')
sys.path.insert(0, '/opt/pypackages')

import numpy as np
import ml_dtypes

BF16 = ml_dtypes.bfloat16

N_NODES = 100000
NCORES = 8
NB = 100                            # dst blocks per core
PER_CORE = NB * 128                 # 12800
N_PAD = NCORES * PER_CORE           # 102400
NQ = 4                              # src chunks (table quarters)
QROWS = PER_CORE // NQ              # 3200 rows each core contributes per chunk
CHUNK = NCORES * QROWS              # 25600 rows per gathered chunk
HID = 128
X_DIM = 4
Y_DIM = 2
IN_DIM = X_DIM + Y_DIM
GPC = 8                             # L2 groups per gather call (64 desc/engine)
IB = 8                              # L2 groups per indicator build
SINGLE_PACKET = True                # 64 descriptors/engine per packet is the max


# ----------------------------------------------------------------- host prep

def _rank_in_cell(cell_id, n_cells):
    order = np.argsort(cell_id, kind='stable')
    cs = np.bincount(cell_id, minlength=n_cells)
    starts = np.zeros_like(cs)
    starts[1:] = np.cumsum(cs)[:-1]
    rank = np.arange(cell_id.shape[0], dtype=np.int64) - np.repeat(starts, cs)
    return order, rank


def _prep(x, edge_index, y, batch):
    """Bucket edges, build per-core device arrays. Returns (G, Q, in_maps)."""
    src = np.asarray(edge_index[0], dtype=np.int64)
    dst = np.asarray(edge_index[1], dtype=np.int64)

    deg = np.bincount(dst, minlength=N_NODES).astype(np.float64) + 1.0
    dinv = (1.0 / np.sqrt(deg)).astype(np.float32)
    dinv_pad = np.zeros(N_PAD, np.float32)
    dinv_pad[:N_NODES] = dinv

    y_node = np.asarray(y)[np.asarray(batch)]                # [N, Y_DIM]
    h0 = np.concatenate([np.asarray(x), y_node], axis=1).astype(np.float32)

    # ---- L1 streams: edges + self loops, norm folded into the message
    loop = np.arange(N_NODES, dtype=np.int64)
    src1 = np.concatenate([src, loop])
    dst1 = np.concatenate([dst, loop])
    norm1 = (dinv[src1] * dinv[dst1]).astype(np.float32)
    c1 = dst1 // PER_CORE
    blk1 = (dst1 % PER_CORE) // 128
    slot1 = (dst1 % 128).astype(np.float32)
    cell1 = c1 * NB + blk1
    cnt1 = np.bincount(cell1, minlength=NCORES * NB)
    G = int(-(-cnt1.max() // 128))
    CAP1 = G * 128
    order1, rank1 = _rank_in_cell(cell1, NCORES * NB)
    pos1 = cell1[order1] * CAP1 + rank1
    hsv = np.zeros((NCORES * NB * CAP1, IN_DIM), np.float32)
    hsv[pos1] = h0[src1[order1]] * norm1[order1][:, None]
    sl1 = np.full(NCORES * NB * CAP1, -1.0, np.float32)
    sl1[pos1] = slot1[order1]
    hsv = hsv.reshape(NCORES, NB * G, 128, IN_DIM).astype(BF16)
    sl1 = sl1.reshape(NCORES, NB * G, 128)

    # ---- L2 gather streams: original edges by (dst core, dst block, src q)
    c2 = dst // PER_CORE
    blk2 = (dst % PER_CORE) // 128
    slot2 = (dst % 128).astype(np.float32)
    q2 = (src % PER_CORE) // QROWS
    loc2 = (src // PER_CORE) * QROWS + (src % PER_CORE) % QROWS   # < 25600
    cell2 = (c2 * NB + blk2) * NQ + q2
    cnt2 = np.bincount(cell2, minlength=NCORES * NB * NQ)
    Q = int(-(-cnt2.max() // 128))
    CELL2 = Q * 128
    NG2 = NB * Q
    order2, rank2 = _rank_in_cell(cell2, NCORES * NB * NQ)
    pos2 = cell2[order2] * CELL2 + rank2
    loc_pad = np.zeros(NCORES * NB * NQ * CELL2, np.int16)
    loc_pad[pos2] = loc2[order2].astype(np.int16)
    slot_pad = np.full(NCORES * NB * NQ * CELL2, -1.0, np.float32)
    slot_pad[pos2] = slot2[order2]
    loc_pad = loc_pad.reshape(NCORES, NB, NQ, CELL2)
    slot_pad = slot_pad.reshape(NCORES, NB, NQ, CELL2)

    iota = np.tile(np.arange(128, dtype=np.float32), (128, 1)).astype(BF16)
    ident = np.eye(128, dtype=np.float32).astype(BF16)

    in_maps = []
    for c in range(NCORES):
        lo = c * PER_CORE
        dv = dinv_pad[lo:lo + PER_CORE]
        m = dict(
            hsT=np.ascontiguousarray(
                hsv[c].transpose(1, 0, 2).reshape(128, NB * G * IN_DIM)),
            dstloc1=np.ascontiguousarray(sl1[c].T).astype(BF16),  # [128, NB*G]
            dinv_col=np.ascontiguousarray(dv.reshape(NB, 128).T), # [128, NB]
            dinv_bcast=np.tile(dv, (128, 1)).astype(BF16),        # [128, 12800]
            iota=iota, ident=ident,
        )
        # dstloc2: [128, NQ, NG2]  (partition = edge pos in group)
        dl = slot_pad[c].transpose(1, 0, 2).reshape(NQ, NG2, 128)
        m['dstloc2'] = np.ascontiguousarray(dl.transpose(2, 0, 1)).astype(BF16)
        # idx streams packed [16, NG2*8] -> replicate to [128, NG2*8]
        for q in range(NQ):
            st = loc_pad[c, :, q, :].reshape(-1)                  # [NG2*128]
            packed = np.ascontiguousarray(
                st.reshape(NG2 * 8, 16).T)                        # [16, NG2*8]
            m[f'idx{q}'] = np.tile(packed, (8, 1))
        in_maps.append(m)
    return G, Q, in_maps


# --------------------------------------------------------------- bass build

_CACHE = {}


def _build(G, Q):
    key = (G, Q)
    if key in _CACHE:
        return _CACHE[key]
    import concourse.bass as bass
    import concourse.bacc as bacc
    import concourse.mybir as mybir
    import concourse.tile as tile
    from concourse.alu_op_type import AluOpType

    f32 = mybir.dt.float32
    bf16 = mybir.dt.bfloat16
    i16 = mybir.dt.int16
    RELU = mybir.ActivationFunctionType.Relu
    IDENT = mybir.ActivationFunctionType.Identity

    NG2 = NB * Q
    IBL1 = -(-G // 2)                   # L1 indicator builds: 2 per block

    nc = bacc.Bacc("TRN2", target_bir_lowering=False, debug=False,
                   enable_asserts=False, num_devices=NCORES,
                   num_swdge_queues=4)

    ins = {}
    for name, shape, dt in [
        ('hsT', [128, NB * G * IN_DIM], bf16),
        ('dstloc1', [128, NB * G], bf16),
        ('dstloc2', [128, NQ, NG2], bf16),
        ('dinv_col', [128, NB], f32),
        ('dinv_bcast', [128, PER_CORE], bf16),
        ('iota', [128, 128], bf16), ('ident', [128, 128], bf16),
        ('W1b', [IN_DIM, HID], bf16), ('W2b', [HID, HID], bf16),
        ('W3b', [HID, HID], bf16), ('W4b', [HID, 8], bf16),
        ('b1c', [128, 1], f32), ('b2c', [128, 1], f32),
        ('b3c', [128, 1], f32), ('b4c', [8, 1], f32),
    ] + [(f'idx{q}', [128, NG2 * 8], i16) for q in range(NQ)]:
        ins[name] = nc.dram_tensor(name, shape, dt, kind="ExternalInput").ap()

    st_out = nc.dram_tensor("st", [8, PER_CORE], f32, kind="ExternalOutput").ap()

    with tile.TileContext(nc) as tc:
        with (
            tc.tile_pool(name="const", bufs=1) as cpool,
            tc.tile_pool(name="msg", bufs=12) as mpool,
            tc.tile_pool(name="ind", bufs=6) as ipool,
            tc.tile_pool(name="ind1", bufs=4) as i1pool,
            tc.tile_pool(name="tb", bufs=12) as tpool,
            tc.tile_pool(name="p6", bufs=2, space="PSUM") as p6pool,
            tc.tile_pool(name="pp", bufs=2, space="PSUM") as ppool,
            tc.tile_pool(name="acc", bufs=2, space="PSUM") as apool,
            tc.tile_pool(name="ph", bufs=2, space="PSUM") as hpool,
            tc.tile_pool(name="dram", bufs=1, space="DRAM") as dpool,
        ):
            # ---- resident constants
            C = {}
            for name in ins:
                ap = ins[name]
                t = cpool.tile(list(ap.shape), ap.dtype, name=f'c_{name}')
                nc.sync.dma_start(t[:], ap)
                C[name] = t

            ag_in = dpool.tile([PER_CORE, HID], bf16, name='ag_in')
            ag_out = [dpool.tile([CHUNK, HID], bf16, addr_space="Shared",
                                 name=f'ag_out{q}') for q in range(NQ)]

            # ---- layer 1: stream host-gathered norm*h0[src], DVE onehots
            for b in range(NB):
                P = p6pool.tile([IN_DIM, 128], f32, name='P', tag='P')
                it1 = None
                i1g = 0
                for j in range(G):
                    g = b * G + j
                    if j % IBL1 == 0:
                        n = min(IBL1, G - j)
                        it1 = i1pool.tile([128, n, 128], bf16, name='it1')
                        dl = C['dstloc1'][:, g:g + n]
                        nc.vector.tensor_tensor(
                            it1[:], dl.to_broadcast([128, n, 128]),
                            C['iota'][:].unsqueeze(1)
                                .to_broadcast([128, n, 128]),
                            op=AluOpType.is_equal)
                        i1g = j
                    nc.tensor.matmul(
                        P[:], C['hsT'][:, g * IN_DIM:(g + 1) * IN_DIM],
                        it1[:, j - i1g, :],
                        start=(j == 0), stop=(j == G - 1))
                Pb = tpool.tile([IN_DIM, 128], bf16, name='Pb')
                nc.scalar.copy(Pb[:], P[:])
                acc1 = ppool.tile([128, 128], f32, name='acc1', tag='pp')
                nc.tensor.matmul(acc1[:], C['W1b'][:], Pb[:],
                                 start=True, stop=True)
                hT = tpool.tile([128, 128], bf16, name='hT1')
                nc.scalar.activation(hT[:], acc1[:], RELU, bias=C['b1c'][:])
                pt2 = ppool.tile([128, 128], f32, name='pt2', tag='pp')
                nc.tensor.matmul(pt2[:], hT[:], C['W2b'][:],
                                 start=True, stop=True)
                t2 = tpool.tile([128, 128], bf16, name='t2')
                nc.scalar.mul(t2[:], pt2[:], C['dinv_col'][:, b:b + 1])
                nc.sync.dma_start(ag_in[b * 128:(b + 1) * 128, :], t2[:])

            # ---- table quarters -> shared chunks (start as quarters finish)
            for q in range(NQ):
                nc.gpsimd.collective_compute(
                    "AllGather", mybir.AluOpType.bypass,
                    replica_groups=[list(range(NCORES))],
                    ins=[ag_in[q * QROWS:(q + 1) * QROWS, :].opt()],
                    outs=[ag_out[q][:].opt()])

            # ---- layer 2 aggregation + head
            msg_t = [None] * NQ
            ind_t = [None] * NQ
            for b in range(NB):
                selfb = tpool.tile([128, HID], bf16, name='selfb')
                nc.sync.dma_start(selfb[:], ag_in[b * 128:(b + 1) * 128, :])
                acc = apool.tile([128, 128], f32, name='acc', tag='acc')
                nc.tensor.matmul(acc[:], selfb[:], C['ident'][:],
                                 start=True, stop=False)
                for q in range(NQ):
                    for k in range(Q):
                        g = b * Q + k
                        if g % GPC == 0:
                            n = min(GPC, NG2 - g)
                            mt = mpool.tile([128, n, HID], bf16, name='mt')
                            nc.gpsimd.dma_gather(
                                mt[:], ag_out[q][:],
                                C[f'idx{q}'][:, g * 8:(g + n) * 8],
                                num_idxs=n * 128,
                                num_idxs_reg=n * 128, elem_size=HID,
                                single_packet=SINGLE_PACKET,
                                queue_num=q)
                            msg_t[q] = (mt, g)
                        if g % IB == 0:
                            n = min(IB, NG2 - g)
                            it = ipool.tile([128, n, 128], bf16, name='it')
                            dl = C['dstloc2'][:, q, g:g + n]
                            nc.vector.tensor_tensor(
                                it[:], dl.to_broadcast([128, n, 128]),
                                C['iota'][:].unsqueeze(1)
                                    .to_broadcast([128, n, 128]),
                                op=AluOpType.is_equal)
                            ind_t[q] = (it, g)
                        mt, mg = msg_t[q]
                        it, ig = ind_t[q]
                        last = (q == NQ - 1) and (k == Q - 1)
                        nc.tensor.matmul(acc[:], mt[:, g - mg, :],
                                         it[:, g - ig, :],
                                         start=False, stop=last)
                # epilogue: h2 = relu(acc * dinv + b2); head W3/W4
                sT2 = tpool.tile([128, 128], f32, name='sT2')
                nc.vector.tensor_tensor(
                    sT2[:], acc[:],
                    C['dinv_bcast'][:, b * 128:(b + 1) * 128],
                    op=AluOpType.mult)
                hT2 = tpool.tile([128, 128], bf16, name='hT2')
                nc.scalar.activation(hT2[:], sT2[:], RELU, bias=C['b2c'][:])
                pm = hpool.tile([128, 128], f32, name='pm', tag='ph')
                nc.tensor.matmul(pm[:], C['W3b'][:], hT2[:],
                                 start=True, stop=True)
                m1 = tpool.tile([128, 128], bf16, name='m1')
                nc.scalar.activation(m1[:], pm[:], RELU, bias=C['b3c'][:])
                ps = hpool.tile([8, 128], f32, name='ps', tag='ph')
                nc.tensor.matmul(ps[:], C['W4b'][:], m1[:],
                                 start=True, stop=True)
                so = tpool.tile([8, 128], f32, name='so')
                nc.scalar.activation(so[:], ps[:], IDENT, bias=C['b4c'][:])
                nc.sync.dma_start(st_out[:, b * 128:(b + 1) * 128], so[:])

    nc.compile()
    _CACHE[key] = nc
    return nc


# ------------------------------------------------------------------- driver

def _install_profile_hook():
    """Wire antenv.axon_hooks -> ctypes NTFF profile against libaxon_pjrt.so
    (the agent image ships the .so but not the antenv glue)."""
    import sys, types, ctypes, contextlib
    try:
        from antenv.axon_hooks import get_axon_ntff_profile_hook  # noqa
        return True
    except ImportError:
        pass
    so_path = '/opt/axon/libaxon_pjrt.so'
    try:
        lib = ctypes.CDLL(so_path)
    except OSError:
        return False
    if not hasattr(lib, 'axon_start_nrt_profile'):
        return False
    lib.axon_start_nrt_profile.argtypes = [ctypes.POINTER(ctypes.c_int64),
                                           ctypes.c_size_t]
    lib.axon_start_nrt_profile.restype = ctypes.c_int64
    lib.axon_stop_nrt_profile.argtypes = [ctypes.c_char_p]
    lib.axon_stop_nrt_profile.restype = ctypes.c_int64

    @contextlib.contextmanager
    def _hook(output_dir, device_ids):
        import jax
        jax.devices()
        if device_ids:
            ids = (ctypes.c_int64 * len(device_ids))(*device_ids)
            rc = lib.axon_start_nrt_profile(ids, len(device_ids))
        else:
            rc = lib.axon_start_nrt_profile(None, 0)
        if rc != 0:
            raise RuntimeError(f"axon_start_nrt_profile rc={rc}")
        try:
            yield
        finally:
            n = lib.axon_stop_nrt_profile(str(output_dir).encode())
            print(f"profile: {n} ntff file(s) written to {output_dir}")

    mod = types.ModuleType('antenv.axon_hooks')
    _h = [_hook]
    mod.set_axon_ntff_profile_hook = lambda h: _h.__setitem__(0, h)
    mod.get_axon_ntff_profile_hook = lambda: _h[0]
    sys.modules['antenv.axon_hooks'] = mod
    import antenv
    antenv.axon_hooks = mod
    return True


def kernel(x, edge_index, y, batch, W1, b1, W2, b2, W3, b3, W4, b4,
           _trace=False, _tmpdir=None):
    from concourse.bass_utils import run_bass_kernel_spmd

    if _trace:
        _trace = _install_profile_hook()
        if _trace:
            import concourse.bass_utils as _bu
            _bu.upload_artifacts = lambda d: f"local://{d}"

    G, Q, in_maps = _prep(x, edge_index, y, batch)
    consts = dict(
        W1b=np.asarray(W1, np.float32).astype(BF16),
        W2b=np.asarray(W2, np.float32).astype(BF16),
        W3b=np.asarray(W3, np.float32).astype(BF16),
        W4b=np.asarray(W4, np.float32).astype(BF16),
        b1c=np.asarray(b1, np.float32).reshape(128, 1),
        b2c=np.asarray(b2, np.float32).reshape(128, 1),
        b3c=np.asarray(b3, np.float32).reshape(128, 1),
        b4c=np.asarray(b4, np.float32).reshape(8, 1),
    )
    for m in in_maps:
        m.update(consts)

    nc = _build(G, Q)
    res = run_bass_kernel_spmd(nc, in_maps, core_ids=list(range(NCORES)),
                               trace=_trace, tmpdir=_tmpdir)
    st = np.concatenate([res.results[c]['st'] for c in range(NCORES)], axis=1)
    st = st[:, :N_NODES]
    s = np.ascontiguousarray(st[:X_DIM].T)
    t = np.ascontiguousarray(st[X_DIM:].T)
    if _trace:
        kernel._last_results = res
    return (s, t)
